# revision 13
# baseline (speedup 1.0000x reference)
"""MultiHeadAttention Trainium2 Bass kernel (B=8, S=1024, D=1024, H=16).

Sharding: head-parallel with valid_len-clamped work. All 8 cores run ONE
identical SPMD program of n "slots"; slot j processes batch sigma(j) and
core i processes head-pair i (output columns 128i..128i+128) of every
batch. Key positions beyond valid_len are masked to exp(-1e9)=0 in the
reference, so slot j only projects K/V and attends over
c_j = ceil(valid_len/128) kpos chunks. Per-core PE work drops from
8*ceil(S/128) to sum_b c_b chunk-columns and is identical on every core
(perfect balance), because each core sees the same slot structure and
differs only in which 128 W-columns it loads.

Per-slot pipeline (all matmul inputs bf16, PSUM accumulation fp32):
  * Projections: Q^T = W_q-slice^T-stationary x X_q^T-moving -> [128, S];
    K^T likewise over c*128 kpos; V: X_v^T-stationary x W_v-slice-moving
    -> VA [128 kpos, c, 2*65] with a ones column per head (attnV then
    also produces softmax denominators).
  * Scores: per kpos-chunk c / q-half qc: scores^T[kpos, q] =
    K_h-stationary x Q_h^T-moving, two heads packed via tile_position
    row groups (K=64 each, run concurrently in the PE array).
  * Softmax: ONE ScalarE exp per chunk, additive -1e9 mask fused via the
    per-partition bias port; no max-subtraction (scores ~ N(0,1)).
  * attnV: po[65, 512] += VA_h^T x P^T_h accumulated over c in PSUM.
  * Output: PE-transpose [65, 128] blocks, reciprocal of the denominator
    column, per-partition multiply into [S, 128] fp32, DMA out.

Slot j's projections are interleaved into slot j-1's attention chunks,
the out-phase of slot j-1 drains during slot j, and slot j+2's input DMA
issues during slot j — PE stays saturated while ~ (2 + 0.5c) MB/slot
streams at ~380 GB/s.
"""
from collections import deque

import numpy as np
import ml_dtypes

import concourse.bass as bass
import concourse.mybir as mybir
import concourse.tile as tile
from concourse.bass_utils import run_bass_kernel_spmd

F32 = mybir.dt.float32
BF16 = mybir.dt.bfloat16
AF = mybir.ActivationFunctionType

B, S, D, H = 8, 1024, 1024, 16
DH = D // H          # 64
NEG = -1.0e9
N_CORES = 8

_cache = {}


def _split_excess_waits(nc, limit: int = 1):
    """Walrus TPB instruction structs encode exactly ONE wait; hoist excess
    waits emitted by Tile into standalone InstEventSemaphore instructions."""
    ctr = 0
    for f in nc.m.functions:
        for bb in f.blocks:
            new = []
            changed = False
            for inst in bb.instructions:
                si = inst.sync_info
                waits = list(si.on_wait) if si is not None and si.on_wait else []
                if len(waits) > limit:
                    excess, keep = waits[:-limit], waits[-limit:]
                    for w in excess:
                        ctr += 1
                        new.append(mybir.InstEventSemaphore(
                            name=f"wsplit-{ctr}",
                            engine=inst.engine,
                            ins=[], outs=[],
                            sync_info=mybir.SyncInfo(on_wait=[w], on_update=[]),
                        ))
                    inst.sync_info = mybir.SyncInfo(
                        on_wait=keep,
                        on_update=list(si.on_update) if si.on_update else [],
                    )
                    changed = True
                new.append(inst)
            if changed:
                bb.instructions = new
    return ctr


def _chunks(valid_lens):
    """Per-batch kpos chunk counts, clamped to [0, 8]."""
    return [min(8, max(0, -(-int(l) // 128))) for l in valid_lens]


def _slot_order(cc):
    """Slot sequencing for the pipeline:
    - first: a ~medium slot (~4.5 MB of X), so the PE warm-up + first
      projections just cover the DMA stream-in time;
    - then big slots early (their X streams while attention windows are
      long), small (c<=3) slots woven between them so their out-phase
      items drain inside later big windows instead of piling up;
    - last: another ~medium slot so the final out-phase is short."""
    order = [b for b in sorted(range(B), key=lambda b: (-cc[b], b)) if cc[b] > 0]
    if len(order) <= 2:
        return order
    j = min(range(len(order)), key=lambda j: (abs(cc[order[j]] - 5), j))
    first = order.pop(j)
    # rest stays plain-descending: mid-kernel PE-duty dips make the HAM
    # governor down-shift to half clock (measured), so small slots bunch
    # at the end rather than interleaving with big ones.
    return [first] + order


def _build_program(cs):
    """cs: tuple of per-slot kpos chunk counts (all >= 1), slot order fixed."""
    n = len(cs)
    total_c = sum(cs)
    offs = [sum(cs[:j]) for j in range(n)]
    nc = bass.Bass()
    xqs = [nc.declare_dram_parameter(f"xq{j}", [D, S], BF16, isOutput=False)
           for j in range(n)]
    xks = [nc.declare_dram_parameter(f"xk{j}", [D, cs[j] * 128], BF16,
                                     isOutput=False) for j in range(n)]
    xvs = [nc.declare_dram_parameter(f"xv{j}", [D, cs[j] * 128], BF16,
                                     isOutput=False) for j in range(n)]
    wqs = nc.declare_dram_parameter("wqs", [D, 128], BF16, isOutput=False)
    wks = nc.declare_dram_parameter("wks", [D, 128], BF16, isOutput=False)
    wvs = nc.declare_dram_parameter("wvs", [D, 128], BF16, isOutput=False)
    msk = nc.declare_dram_parameter("msk", [128, total_c], F32, isOutput=False)
    idn = nc.declare_dram_parameter("idn", [128, 128], BF16, isOutput=False)
    outs = [nc.declare_dram_parameter(f"out{j}", [S, 128], BF16, isOutput=True)
            for j in range(n)]

    with tile.TileContext(nc) as tc:
        with (
            tc.tile_pool(name="persist", bufs=1) as pers,
            tc.tile_pool(name="xw", bufs=26) as xw,
            tc.tile_pool(name="qkv", bufs=2) as qkv,
            tc.tile_pool(name="pt", bufs=8) as ptp,
            tc.tile_pool(name="outt", bufs=6) as outtp,
            tc.tile_pool(name="outp", bufs=2) as outp,
            tc.tile_pool(name="rr", bufs=8) as rrp,
            tc.tile_pool(name="pp", bufs=2, space="PSUM") as pp,
            tc.tile_pool(name="psc", bufs=2, space="PSUM") as psc,
            tc.tile_pool(name="pod", bufs=2, space="PSUM") as pod,
        ):
            # ---------- constants ----------
            mask_sb = pers.tile([128, total_c], F32)
            nc.sync.dma_start(out=mask_sb, in_=msk[:, :])
            id_sb = pers.tile([128, 128], BF16)
            nc.sync.dma_start(out=id_sb, in_=idn[:, :])
            wq_sb = pers.tile([128, 8, 128], BF16)
            nc.sync.dma_start(out=wq_sb,
                              in_=wqs[:, :].rearrange("(a p) m -> p a m", p=128))
            wk_sb = pers.tile([128, 8, 128], BF16)
            nc.sync.dma_start(out=wk_sb,
                              in_=wks[:, :].rearrange("(a p) m -> p a m", p=128))
            wv_sb = pers.tile([128, 8, 128], BF16)
            nc.sync.dma_start(out=wv_sb,
                              in_=wvs[:, :].rearrange("(a p) m -> p a m", p=128))
            warm = pers.tile([128, 1], F32)
            nc.scalar.copy(warm, mask_sb[:, 0:1])            # warm ACT clock
            pw = pp.tile([32, 32], BF16, tag="pp")
            nc.tensor.transpose(pw[:, :], id_sb[0:32, 0:32], id_sb[0:32, 0:32])
            # HAM warm-up: junk matmuls on the identity tile while input DMAs
            # stream, so the first projection matmuls run at full clock
            for _ in range(4):
                jw = pp.tile([32, 128], F32, tag="pp")
                for j in range(10):
                    nc.tensor.matmul(
                        jw[:, :], lhsT=id_sb[0:32, 0:32],
                        rhs=id_sb[0:32, 0:128],
                        start=(j == 0), stop=(j == 9))

            # ---------- streaming state ----------
            stage = {}    # slot -> {"xq": [4 tiles], "xk": [...], "xv": [...]}
            slotqkv = {}  # slot -> (QT, KT, VA)

            def dma_gen(j):
                e = stage[j] = {"xq": [], "xk": [], "xv": []}
                for q in range(4):
                    t = xw.tile([128, 2, S], BF16, tag="xw", name=f"xq{j}_{q}")
                    nc.sync.dma_start(
                        out=t, in_=xqs[j].rearrange(
                            "(a p) s -> p a s", p=128)[:, 2 * q:2 * q + 2, :])
                    e["xq"].append(t)
                    yield
                w = cs[j] * 128
                for which, prm in (("xk", xks[j]), ("xv", xvs[j])):
                    for q in range(4):
                        t = xw.tile([128, 2, S], BF16, tag="xw",
                                    name=f"{which}{j}_{q}")
                        nc.sync.dma_start(
                            out=t[:, :, 0:w], in_=prm.rearrange(
                                "(a p) s -> p a s", p=128)[:, 2 * q:2 * q + 2, :])
                        e[which].append(t)
                        yield

            def xsl(lst, k):
                return lst[k // 2][:, k % 2, :]

            def proj_gen(j):
                c = cs[j]
                QT = qkv.tile([128, S], BF16, tag="qt", name=f"QT{j}")
                KT = qkv.tile([128, S], BF16, tag="kt", name=f"KT{j}")
                VA = qkv.tile([128, 8, 130], BF16, tag="va", name=f"VA{j}")
                slotqkv[j] = (QT, KT, VA)
                e = stage[j]
                for sc in range(2):
                    pq = pp.tile([128, 512], F32, tag="pp", name=f"pq{j}_{sc}")
                    for k in range(8):
                        nc.tensor.matmul(
                            pq[:, :], lhsT=wq_sb[:, k, :],
                            rhs=xsl(e["xq"], k)[:, bass.ts(sc, 512)],
                            start=(k == 0), stop=(k == 7))
                        yield
                    nc.vector.tensor_copy(QT[:, bass.ts(sc, 512)], pq)
                for g0 in range(0, c * 128, 512):
                    gw = min(512, c * 128 - g0)
                    pk = pp.tile([128, 512], F32, tag="pp", name=f"pk{j}_{g0}")
                    for k in range(8):
                        nc.tensor.matmul(
                            pk[:, 0:gw], lhsT=wk_sb[:, k, :],
                            rhs=xsl(e["xk"], k)[:, g0:g0 + gw],
                            start=(k == 0), stop=(k == 7))
                        yield
                    nc.vector.tensor_copy(KT[:, g0:g0 + gw], pk[:, 0:gw])
                for st in range(c):
                    pv = pp.tile([128, 512], F32, tag="pp", name=f"pv{j}_{st}")
                    for k in range(8):
                        nc.tensor.matmul(
                            pv[:, 0:128],
                            lhsT=xsl(e["xv"], k)[:, bass.ts(st, 128)],
                            rhs=wv_sb[:, k, :],
                            start=(k == 0), stop=(k == 7))
                        yield
                    dst = VA[:, st, :].rearrange("p (h w) -> p h w", w=65)
                    nc.vector.tensor_copy(
                        dst[:, :, 0:64],
                        pv[:, 0:128].rearrange("p (h w) -> p h w", w=64))
                    nc.vector.memset(dst[:, :, 64:65], 1.0)

            def out_gen(j, oT1, oT2, OP, qhalf):
                """Out-phase for q-columns [qhalf*512, qhalf*512+512): those
                oT columns are final right after q-half qhalf's PSUM copy,
                so the first half drains during the SAME slot's second half."""
                for qt in range(4 * qhalf, 4 * qhalf + 4):
                    for hh in range(2):
                        oT = oT1 if hh == 0 else oT2
                        ptr = pp.tile([128, 65], BF16, tag="pp",
                                      name=f"ptr{j}_{qt}_{hh}")
                        nc.tensor.transpose(ptr[:, :], oT[:, bass.ts(qt, 128)],
                                            id_sb[0:65, 0:65])
                        rr = rrp.tile([128, 1], F32, tag="rr")
                        nc.vector.reciprocal(rr, ptr[:, 64:65])
                        nc.vector.tensor_scalar_mul(
                            OP[:, qt, hh * DH:(hh + 1) * DH],
                            ptr[:, 0:64], rr[:, 0:1])
                        yield
                if qhalf == 1:
                    nc.gpsimd.dma_start(
                        out=outs[j][:, :].rearrange("(a p) w -> p a w", p=128),
                        in_=OP[:, :, :])

            def pump(q, k):
                done = 0
                while done < k and q:
                    try:
                        next(q[0])
                        done += 1
                    except StopIteration:
                        q.popleft()
                return done

            def drain(q):
                while q:
                    try:
                        next(q[0])
                    except StopIteration:
                        q.popleft()

            projq, outq, dmaq = deque(), deque(), deque()

            # prelude: issue slots 0,1 loads; late HAM warm-up gated on an
            # xq0 piece that arrives just before the first dense burst
            dmaq.append(dma_gen(0))
            if n > 1:
                dmaq.append(dma_gen(1))
            pump(dmaq, 3)      # xq0 quarters 0-2 issued
            gate = stage[0]["xq"][2]
            jl = pp.tile([32, 128], F32, tag="pp", name="jlate")
            for j in range(6):
                nc.tensor.matmul(
                    jl[:, :], lhsT=gate[0:32, 0, 0:32],
                    rhs=gate[0:32, 0, 0:128],
                    start=(j == 0), stop=(j == 5))
            drain(dmaq)        # rest of slot 0 + slot 1 loads issued

            projq.append(proj_gen(0))
            drain(projq)       # slot 0 projections emitted (arrival-paced)
            if n > 1:
                projq.append(proj_gen(1))

            for j in range(n):
                cj = cs[j]
                if j + 2 < n:
                    dmaq.append(dma_gen(j + 2))
                steps = 2 * cj
                if j + 1 < n:
                    cn = cs[j + 1]
                    m_next = 16 + 8 * ((cn + 3) // 4) + 8 * cn
                    p_pace = -(-m_next // steps)
                else:
                    p_pace = 0
                d_pace = -(-12 // steps)
                QT, KT, VA = slotqkv[j]
                oT1 = outtp.tile([65, S], BF16, tag="outt", name=f"oT1_{j}")
                oT2 = outtp.tile([65, S], BF16, tag="outt", name=f"oT2_{j}")
                OP = outp.tile([128, 8, 128], BF16, tag="outp", name=f"OP{j}")
                moff = offs[j]
                for qc in range(2):
                    po1 = pod.tile([65, 512], F32, tag="pod")
                    po2 = pod.tile([65, 512], F32, tag="pod")
                    for c in range(cj):
                        ps = psc.tile([128, 1024], F32, tag="psc")
                        nc.tensor.matmul(
                            ps[:, 0:512],
                            lhsT=KT[0:64, bass.ts(c, 128)],
                            rhs=QT[0:64, bass.ts(qc, 512)],
                            start=True, stop=True, tile_position=(0, 0))
                        nc.tensor.matmul(
                            ps[:, 512:1024],
                            lhsT=KT[64:128, bass.ts(c, 128)],
                            rhs=QT[64:128, bass.ts(qc, 512)],
                            start=True, stop=True, tile_position=(64, 0))
                        pt = ptp.tile([128, 1024], BF16, tag="pt")
                        nc.scalar.activation(pt, ps, AF.Exp,
                                             bias=mask_sb[:, moff + c:moff + c + 1],
                                             scale=1.0)
                        nc.tensor.matmul(
                            po1[:, :], lhsT=VA[:, c, 0:65],
                            rhs=pt[:, 0:512],
                            start=(c == 0), stop=(c == cj - 1))
                        nc.tensor.matmul(
                            po2[:, :], lhsT=VA[:, c, 65:130],
                            rhs=pt[:, 512:1024],
                            start=(c == 0), stop=(c == cj - 1))
                        if c < cj - 1:
                            pump(projq, p_pace)
                            pump(outq, 4)
                            pump(dmaq, d_pace)
                    # oT copies first: out-phase items queued ahead of them
                    # on the Vector engine would delay the next q-half's
                    # attnV through the pod PSUM ring
                    nc.vector.tensor_copy(oT1[:, bass.ts(qc, 512)], po1)
                    nc.vector.tensor_copy(oT2[:, bass.ts(qc, 512)], po2)
                    pump(projq, p_pace)
                    pump(outq, 4)
                    pump(dmaq, d_pace)
                    outq.append(out_gen(j, oT1, oT2, OP, qc))
                pump(outq, 6)
                drain(projq)          # finish next slot's projections
                if j + 2 < n:
                    projq.append(proj_gen(j + 2))
                drain(dmaq)

            drain(outq)

    _split_excess_waits(nc)
    return nc


def _prep_inputs(queries, keys, values, valid_lens, w_q, w_k, w_v):
    """Returns per-core in_maps for the slot program of these valid_lens."""
    bf = ml_dtypes.bfloat16
    cc = _chunks(valid_lens)
    order = _slot_order(cc)
    cs = [cc[b] for b in order]
    scale = 1.0 / np.sqrt(DH)
    idn = np.eye(128, dtype=bf)

    shared = {"idn": idn}
    mcols = []
    for j, b in enumerate(order):
        c = cs[j]
        shared[f"xq{j}"] = np.ascontiguousarray(
            queries[b].astype(np.float32).T.astype(bf))
        shared[f"xk{j}"] = np.ascontiguousarray(
            keys[b].astype(np.float32).T[:, :c * 128].astype(bf))
        shared[f"xv{j}"] = np.ascontiguousarray(
            values[b].astype(np.float32).T[:, :c * 128].astype(bf))
        m = np.where(np.arange(c * 128) < int(valid_lens[b]), 0.0, NEG)
        mcols.append(m.reshape(c, 128).T.astype(np.float32))
    shared["msk"] = np.ascontiguousarray(np.concatenate(mcols, axis=1)) \
        if mcols else np.zeros((128, 0), np.float32)

    wq_f = w_q.astype(np.float32) * scale
    wk_f = w_k.astype(np.float32)
    wv_f = w_v.astype(np.float32)
    in_maps = []
    for i in range(N_CORES):
        sl = slice(128 * i, 128 * (i + 1))
        in_maps.append(dict(
            shared,
            wqs=np.ascontiguousarray(wq_f[:, sl].astype(bf)),
            wks=np.ascontiguousarray(wk_f[:, sl].astype(bf)),
            wvs=np.ascontiguousarray(wv_f[:, sl].astype(bf)),
        ))
    return in_maps


def kernel(queries, keys, values, valid_lens, w_q, w_k, w_v, _want_results=False):
    queries = np.asarray(queries)
    keys = np.asarray(keys)
    values = np.asarray(values)
    valid_lens = np.asarray(valid_lens)
    w_q, w_k, w_v = np.asarray(w_q), np.asarray(w_k), np.asarray(w_v)

    cc = _chunks(valid_lens)
    order = _slot_order(cc)
    cs = tuple(cc[b] for b in order)
    out = np.empty((B, S, D), np.float32)

    if cs:
        if _cache.get("cs") != cs:
            _cache["cs"] = cs
            _cache["nc"] = _build_program(cs)
        nc = _cache["nc"]
        in_maps = _prep_inputs(queries, keys, values, valid_lens, w_q, w_k, w_v)
        res = run_bass_kernel_spmd(nc, in_maps, list(range(N_CORES)))
        for j, b in enumerate(order):
            out[b] = np.concatenate(
                [res.results[i][f"out{j}"] for i in range(N_CORES)], axis=1)
    else:
        res = None

    # valid_len == 0: reference softmaxes an all -1e9 row -> uniform
    # attention = mean of V rows; mean commutes with the projection.
    for b in range(B):
        if cc[b] == 0:
            vbar = values[b].astype(np.float32).mean(axis=0) @ w_v.astype(np.float32)
            out[b] = np.broadcast_to(vbar, (S, D))

    if _want_results:
        return out, res
    return out
